# revision 1
# baseline (speedup 1.0000x reference)
"""Trainium2 Bass kernel for ChunkedLocalSelfAttention.

Module: x[B,C,H,W] -> qkv proj -> 8-head local-window attention (17x17
spatial window) -> out proj -> +residual -> 1x1 conv -> relu.
B,C,H,W = 4,256,48,48; N = 2304 tokens per image; head dim 32.

Sharding: 8 cores = 4 batch images x 2 query-row-halves (24 rows each).
Each core computes the full pipeline for its half-image: attention output
rows only depend on +-8 image rows, so cores need no communication; the
row halo is covered by computing k/v for a 32-row band.

On-core design (scores kept TRANSPOSED: keys on partitions, queries free):
  - qk projection: qkT [512, 2304] = WqkT.T @ xT, bf16
  - v in [token, channel] layout: v = xT.T @ WvT for the 32-row band
  - per query-tile (8 image rows = 384 queries) and head-group (4 heads):
      for each of 9 key-chunks (128 contiguous tokens of the 24-row region):
        scoresT[k=128, h*512+q] via 4 row-packed K=32 matmuls
        exp on ScalarE (scale=1/sqrt(32) fused; logits are tiny so no
        max-subtraction is needed), multiplicative binary window mask on
        VectorE (mask precomputed on host, shared across heads)
        PV += v_chunk.T @ masked  (4 col-packed matmuls, M=32)
        sums += ones.T @ masked   (4 col-packed matmuls, M=32, the ones
                                   lhsT replicates each head's sum into
                                   its 32-partition strip)
      oT = PV * reciprocal(sums) -> bf16
  - out proj, +residual(+out_b folded on host), 1x1 conv, relu+conv_b.
"""

import sys

for _p in ("/opt/trn_rl_repo",):
    if _p not in sys.path:
        sys.path.insert(0, _p)

import math

import ml_dtypes
import numpy as np

B, C, H, W = 4, 256, 48, 48
N = H * W
HEADS, HD, HALF = 8, 32, 8
NCORES = 8
ROWS_HALF = H // 2          # 24 query rows per core
NQ = ROWS_HALF * W          # 1152 queries per core
REG_ROWS = 24               # key-region rows per query tile
NK = REG_ROWS * W           # 1152 keys per region = 9 chunks of 128
NCHUNK = NK // 128          # 9
BAND_ROWS = 32              # k/v row band per core (24 + 8 halo)
QT = 384                    # queries per tile (8 image rows)
SCALE = 1.0 / math.sqrt(HD)

bf16 = ml_dtypes.bfloat16

_PROG = None


def _build_program():
    import concourse.bass as bass
    import concourse.mybir as mybir
    import concourse.tile as tile
    from concourse import bacc

    f32 = mybir.dt.float32
    bft = mybir.dt.bfloat16
    AF = mybir.ActivationFunctionType
    OP = mybir.AluOpType

    nc = bacc.Bacc(
        "TRN2", target_bir_lowering=False, debug=False, num_devices=NCORES
    )

    def din(name, shape, dt=bft):
        return nc.dram_tensor(name, shape, dt, kind="ExternalInput").ap()

    xt_d = din("xT", [C, N])
    xres_d = din("xres", [C, NQ], f32)
    wqk_d = din("wqkT", [C, 2 * C])
    wv_d = din("wvT", [C, C])
    wo_d = din("woT", [C, C])
    wc_d = din("wcT", [C, C])
    bqk_d = din("bqk", [128, 4], f32)
    bv_d = din("bvrep", [128, C], f32)
    bc_d = din("bcrep", [128, 2], f32)
    mask_d = din("masks", [24, 128, QT])
    out_d = nc.dram_tensor("out", [C, NQ], f32, kind="ExternalOutput").ap()

    # SPMD trick: one program must serve both row-halves. The host ships
    # half-1 images VERTICALLY FLIPPED (attention is equivariant under a
    # row flip; the window test is |dh|<=8), so every core sees half-0
    # geometry: query rows [0, 24), key band rows [0, 32). Query tile qt
    # has rows [8qt, 8qt+8) and key-region rows [rs, rs+24), rs={0,0,8}.
    # For qt=0 the last 3 region chunks (rows 16-24) are fully outside the
    # +-8 row window of its queries and are skipped entirely.

    with tile.TileContext(nc) as tc:
        import contextlib

        ctx = contextlib.ExitStack()
        with ctx:
            cpool = ctx.enter_context(tc.tile_pool(name="const", bufs=1))
            qkpool = ctx.enter_context(tc.tile_pool(name="qk", bufs=1))
            vpool = ctx.enter_context(tc.tile_pool(name="v", bufs=1))
            epool = ctx.enter_context(tc.tile_pool(name="exp", bufs=4))
            apool = ctx.enter_context(tc.tile_pool(name="attn", bufs=4))
            rpool = ctx.enter_context(tc.tile_pool(name="recip", bufs=3))
            opool = ctx.enter_context(tc.tile_pool(name="outb", bufs=3))
            psA = ctx.enter_context(
                tc.tile_pool(name="psA", bufs=2, space="PSUM")
            )
            psB = ctx.enter_context(
                tc.tile_pool(name="psB", bufs=2, space="PSUM")
            )

            # ---- constants / inputs to SBUF ----
            xt = [cpool.tile([128, N], bft, tag=f"xt{t}", name=f"xt{t}") for t in range(2)]
            for t in range(2):
                nc.sync.dma_start(xt[t][:], xt_d[128 * t : 128 * t + 128, :])
            wqk = [cpool.tile([128, 2 * C], bft, tag=f"wqk{t}", name=f"wqk{t}") for t in range(2)]
            wv = [cpool.tile([128, C], bft, tag=f"wv{t}", name=f"wv{t}") for t in range(2)]
            wo = [cpool.tile([128, C], bft, tag=f"wo{t}", name=f"wo{t}") for t in range(2)]
            wc = [cpool.tile([128, C], bft, tag=f"wc{t}", name=f"wc{t}") for t in range(2)]
            for t in range(2):
                sl = slice(128 * t, 128 * t + 128)
                nc.sync.dma_start(wqk[t][:], wqk_d[sl, :])
                nc.sync.dma_start(wv[t][:], wv_d[sl, :])
                nc.sync.dma_start(wo[t][:], wo_d[sl, :])
                nc.sync.dma_start(wc[t][:], wc_d[sl, :])
            bqk = cpool.tile([128, 4], f32, tag="bqk")
            bvr = cpool.tile([128, C], f32, tag="bvr")
            bcr = cpool.tile([128, 2], f32, tag="bcr")
            nc.sync.dma_start(bqk[:], bqk_d[:])
            nc.sync.dma_start(bvr[:], bv_d[:])
            nc.sync.dma_start(bcr[:], bc_d[:])
            zrow = cpool.tile([1, 512], bft, tag="zrow")
            nc.vector.memset(zrow[:], 0.0)
            msk = cpool.tile([128, 24 * QT], bft, tag="msk")
            nc.sync.dma_start(
                msk[:].rearrange("p (c q) -> p c q", q=QT),
                mask_d[:].transpose([1, 0, 2]),
            )
            xres = [cpool.tile([128, NQ], f32, tag=f"xres{t}", name=f"xres{t}") for t in range(2)]
            for t in range(2):
                nc.sync.dma_start(xres[t][:], xres_d[128 * t : 128 * t + 128, :])

            # ---- phase 1: qk projection  qkT[512, N] bf16 ----
            # q needed for tokens [0, 1152) only; k for the band [0, 1536)
            qk = [qkpool.tile([128, 1536], bft, tag=f"qk{i}", name=f"qk{i}") for i in range(4)]
            NT_Q = [(0, 384), (384, 384), (768, 384)]
            NT_K = [(0, 512), (512, 512), (1024, 512)]

            def qk_proj(qc):
                for n0, nw in (NT_Q if qc < 2 else NT_K):
                    ps = psB.tile([128, 512], f32, tag="ps", name="ps")
                    for cc in range(2):
                        nc.tensor.matmul(
                            ps[:, :nw],
                            lhsT=wqk[cc][:, 128 * qc : 128 * qc + 128],
                            rhs=xt[cc][:, n0 : n0 + nw],
                            start=(cc == 0),
                            stop=(cc == 1),
                        )
                    nc.vector.tensor_scalar_add(
                        qk[qc][:, n0 : n0 + nw], ps[:, :nw], bqk[:, qc : qc + 1]
                    )

            # v band, token-major, rows [0, 32) -> 12 tiles; layout per
            # tile: head h cols [64h, 64h+32) = v_h, [64h+32, 64h+64) = 1.0
            vt = [vpool.tile([128, 8 * 64], bft, tag=f"v{i}", name=f"v{i}") for i in range(12)]

            def v_proj(i):
                n0 = 128 * i
                ps = psB.tile([128, 512], f32, tag="ps", name="ps")
                for cc in range(2):
                    nc.tensor.matmul(
                        ps[:, :C],
                        lhsT=xt[cc][:, n0 : n0 + 128],
                        rhs=wv[cc][:],
                        start=(cc == 0),
                        stop=(cc == 1),
                    )
                va = vt[i][:].rearrange("p (h two v) -> p h two v", two=2, v=32)
                nc.vector.tensor_add(
                    va[:, :, 0, :],
                    ps[:, :C].rearrange("p (h v) -> p h v", v=32),
                    bvr[:].rearrange("p (h v) -> p h v", v=32),
                )
                nc.gpsimd.memset(va[:, :, 1, :], 1.0)

            # heads 0-3 inputs first so attention can start early
            qk_proj(0)
            qk_proj(2)
            for i in range(6):
                v_proj(i)
            qk_proj(1)
            qk_proj(3)
            for i in range(6, 12):
                v_proj(i)

            # ---- phase 3: attention ----
            # rolled coords: query rows [8, 32): qtile qt rows r0 = 8+8*qt,
            # region rows rs = 8*qt, region tokens [rs*48, rs*48+1152).
            oT = [cpool.tile([128, NQ], bft, tag=f"oT{g}", name=f"oT{g}") for g in range(2)]
            res = [cpool.tile([128, NQ], bft, tag=f"res{t}", name=f"res{t}") for t in range(2)]
            mbase = 0
            for qt in range(3):
                r0 = 8 * qt
                rs = (0, 0, 8)[qt]
                nchunk = (6, 9, 9)[qt]
                q0 = r0 * W
                for g in range(4):
                    # pair tile pp: rows = [pv_h | sums_h | pv_h' | sums_h']
                    # for heads (2g, 2g+1). Col-packed accumulation: two
                    # M=64 matmuls share the bank, so start=True (bank-wide
                    # has_written clear) is unusable; zero the bank and
                    # accumulate from the first matmul.
                    # zero-matmul opens the accumulation group: start=True
                    # clears the bank's has_written and writes zeros to every
                    # element, so the PV matmuls below accumulate from zero.
                    pp = psB.tile([128, QT], f32, tag="pp", name="pp", bufs=2)
                    nc.tensor.matmul(
                        pp[:],
                        lhsT=zrow[:, 0:128],
                        rhs=zrow[:, 0:QT],
                        start=True,
                        stop=False,
                        skip_group_check=True,
                    )
                    for ck in range(nchunk):
                        kof = rs * W + 128 * ck
                        # only queries within +-8 rows of this chunk's keys
                        # participate; the range is 128-token aligned.
                        a = max(kof - 384 - QT * qt, 0)
                        b = min(kof + 512 - QT * qt, QT)
                        qw_ = b - a
                        sc = psA.tile([128, 1024], f32, tag="sc", name="sc")
                        for hh in range(2):
                            h = 2 * g + hh
                            qtile_idx, krow = h // 4, 32 * (h % 4)
                            nc.tensor.matmul(
                                sc[:, 512 * hh + a : 512 * hh + b],
                                lhsT=qk[2 + qtile_idx][
                                    krow : krow + 32, kof : kof + 128
                                ],
                                rhs=qk[qtile_idx][
                                    krow : krow + 32, q0 + a : q0 + b
                                ],
                                start=True,
                                stop=True,
                                tile_position=(krow, 0),
                            )
                        ex = epool.tile([128, 2 * QT], bft, tag="ex", name="ex")
                        sc_v = sc[:].rearrange("p (h q) -> p h q", q=512)[
                            :, :, a:b
                        ]
                        ex_v = ex[:].rearrange("p (h q) -> p h q", q=QT)[
                            :, :, a:b
                        ]
                        nc.scalar.activation(ex_v, sc_v, AF.Exp, scale=SCALE)
                        ma = apool.tile([128, 2 * QT], bft, tag="ma", name="ma")
                        ma_v = ma[:].rearrange("p (h q) -> p h q", q=QT)[
                            :, :, a:b
                        ]
                        mk = msk[:, (mbase + ck) * QT + a : (mbase + ck) * QT + b]
                        nc.vector.tensor_mul(
                            ma_v, ex_v, mk[:, None, :].broadcast_to([128, 2, qw_])
                        )
                        vi = vt[(rs * W + 128 * ck) // 128]
                        for hh in range(2):
                            h = 2 * g + hh
                            nc.tensor.matmul(
                                pp[64 * hh : 64 * hh + 64, a:b],
                                lhsT=vi[:, 64 * h : 64 * h + 64],
                                rhs=ma[:, QT * hh + a : QT * hh + b],
                                start=False,
                                stop=(ck == nchunk - 1 and hh == 1),
                                skip_group_check=True,
                                tile_position=(0, 64 * hh),
                            )
                    # rows of pp: 0-31 pv_a, 32-63 sums_a, 64-95 pv_b, 96-127 sums_b
                    rc = rpool.tile([128, QT], f32, tag="rc", name="rc")
                    nc.vector.reciprocal(rc[:], pp[:])
                    # shift recip(sums) down 32 partitions onto pv lanes
                    rcs = rpool.tile([128, QT], f32, tag="rcs", name="rcs")
                    nc.sync.dma_start(rcs[0:96, :], rc[32:128, :])
                    on = rpool.tile([128, QT], bft, tag="on", name="on")
                    nc.vector.tensor_mul(on[0:96, :], pp[0:96, :], rcs[0:96, :])
                    # compact pv rows {0-31, 64-95} into channel order
                    nc.sync.dma_start(
                        oT[g // 2][
                            64 * (g % 2) : 64 * (g % 2) + 32,
                            QT * qt : QT * qt + QT,
                        ],
                        on[0:32, :],
                    )
                    nc.sync.dma_start(
                        oT[g // 2][
                            64 * (g % 2) + 32 : 64 * (g % 2) + 64,
                            QT * qt : QT * qt + QT,
                        ],
                        on[64:96, :],
                    )
                # ---- projections for this qtile's columns ----
                n0 = QT * qt
                for oc in range(2):
                    ps = psB.tile([128, 512], f32, tag="ps", name="ps")
                    for cc in range(2):
                        nc.tensor.matmul(
                            ps[:, :QT],
                            lhsT=wo[cc][:, 128 * oc : 128 * oc + 128],
                            rhs=oT[cc][:, n0 : n0 + QT],
                            start=(cc == 0),
                            stop=(cc == 1),
                        )
                    nc.vector.tensor_add(
                        res[oc][:, n0 : n0 + QT],
                        ps[:, :QT],
                        xres[oc][:, n0 : n0 + QT],
                    )
                for oc in range(2):
                    ps = psB.tile([128, 512], f32, tag="ps", name="ps")
                    for cc in range(2):
                        nc.tensor.matmul(
                            ps[:, :QT],
                            lhsT=wc[cc][:, 128 * oc : 128 * oc + 128],
                            rhs=res[cc][:, n0 : n0 + QT],
                            start=(cc == 0),
                            stop=(cc == 1),
                        )
                    ob = opool.tile([128, QT], f32, tag="ob", name="ob")
                    nc.vector.tensor_scalar(
                        ob[:],
                        ps[:, :QT],
                        bcr[:, oc : oc + 1],
                        0.0,
                        OP.add,
                        OP.max,
                    )
                    nc.sync.dma_start(
                        out_d[128 * oc : 128 * oc + 128, n0 : n0 + QT], ob[:]
                    )
                mbase += nchunk

    nc.compile()
    return nc


def _get_program():
    global _PROG
    if _PROG is None:
        _PROG = _build_program()
    return _PROG


def _prep_core_inputs(core, x, in_proj_w, in_proj_b, out_w, out_b, conv_w, conv_b):
    b, half = core // 2, core % 2
    ximg = x[b].reshape(C, H, W)
    if half == 1:
        ximg = ximg[:, ::-1, :]  # row-flip: half-1 becomes half-0 geometry
    xres = (ximg[:, :ROWS_HALF, :].reshape(C, NQ) + out_b[:, None]).astype(
        np.float32
    )
    return {
        "xT": np.ascontiguousarray(ximg.reshape(C, N)).astype(bf16),
        "xres": xres,
        "wqkT": np.ascontiguousarray(in_proj_w[: 2 * C].T).astype(bf16),
        "wvT": np.ascontiguousarray(in_proj_w[2 * C :].T).astype(bf16),
        "woT": np.ascontiguousarray(out_w.T).astype(bf16),
        "wcT": np.ascontiguousarray(conv_w.T).astype(bf16),
        "bqk": np.ascontiguousarray(
            in_proj_b[: 2 * C].reshape(4, 128).T
        ).astype(np.float32),
        "bvrep": np.broadcast_to(in_proj_b[2 * C :], (128, C)).astype(np.float32).copy(),
        "bcrep": np.ascontiguousarray(conv_b.reshape(2, 128).T).astype(np.float32),
        "masks": _masks(),
    }


_MASK_CACHE = {}


def _masks() -> np.ndarray:
    """[24, 128, 384] binary window masks, shared by every core.

    Half-0 geometry: qtile qt queries rows [8qt, 8qt+8), region rows
    [rs, rs+24) with rs = (0, 0, 8)[qt]; qt=0 keeps only chunks 0-5.
    """
    if "m" in _MASK_CACHE:
        return _MASK_CACHE["m"]
    outs = []
    for qt, (rs, nchunk) in enumerate(zip((0, 0, 8), (6, 9, 9))):
        r0 = 8 * qt
        qidx = r0 * W + np.arange(QT)
        qh, qw = qidx // W, qidx % W
        for ck in range(nchunk):
            kidx = rs * W + 128 * ck + np.arange(128)
            kh, kw = kidx // W, kidx % W
            m = (np.abs(kh[:, None] - qh[None, :]) <= HALF) & (
                np.abs(kw[:, None] - qw[None, :]) <= HALF
            )
            outs.append(m)
    res = np.stack(outs).astype(bf16)
    _MASK_CACHE["m"] = res
    return res


def kernel(**inputs):
    from concourse.bass_utils import run_bass_kernel_spmd

    args = {k: np.asarray(v) for k, v in inputs.items()}
    nc = _get_program()
    in_maps = [
        _prep_core_inputs(core, **args) for core in range(NCORES)
    ]
    res = run_bass_kernel_spmd(nc, in_maps, core_ids=list(range(NCORES)))
    out = np.zeros((B, C, H, W), np.float32)
    for core in range(NCORES):
        b, half = core // 2, core % 2
        o = res.results[core]["out"].reshape(C, ROWS_HALF, W)
        if half == 1:
            o = o[:, ::-1, :]  # undo the row flip
            out[b][:, ROWS_HALF:, :] = o
        else:
            out[b][:, :ROWS_HALF, :] = o
    return out



# revision 8
# speedup vs baseline: 1.1229x; 1.1229x over previous
"""Trainium2 Bass kernel for ChunkedLocalSelfAttention.

Module: x[B,C,H,W] -> qkv proj -> 8-head local-window attention (17x17
spatial window) -> out proj -> +residual -> 1x1 conv -> relu.
B,C,H,W = 4,256,48,48; per-core half-image: 1152 queries, head dim 32.

Sharding: 8 cores = 4 batch images x 2 query-row-halves (24 rows each).
Attention rows depend only on +-8 image rows, so a 32-row k/v band per
core needs no cross-core traffic. Half-1 images ship vertically flipped
(window test is |dh|<=8, flip-equivariant) so one SPMD program serves
both halves.

On-core design (W-MAJOR token order: q token = 24*w + h, band token =
32*w + h). W-major makes the fine-grained mask axis the dense 24/32-row
axis (17-wide window ~ 50% dense) and the 48-col axis structurally
trimmed at chunk granularity -> ~29% fewer score elements than h-major.

  - scoresT[key,q] per (qblock of 128 q, chunk of 128 keys = 4 w-cols):
    8 head matmuls, K=32 row-packed via tile_position; q-range trimmed
    to the chunk's +-8 w-col reach (32-aligned for free).
  - exp on ScalarE (scale fused; logits tiny, no max-subtraction), one
    instr per pair covering all 8 heads [128, 8, width].
  - binary 2D window mask multiply on VectorE/GpSimd (host precomputed).
  - FLIPPED PV: masked probs are the STATIONARY operand, [v | 1] the
    moving one -> 33 output cols per (pair, head) instead of width.
    PSUM accumulates [128 q, 8*(32 pv + 1 sum)] per qblock (zero-matmul
    opens the bank).
  - normalize per qblock (recip of sums col, broadcast multiply), PE
    transpose to channel-major, DMA psum->sbuf, then out proj,
    +residual (out_b folded host-side), 1x1 conv, relu+conv_b.
"""

import sys

for _p in ("/opt/trn_rl_repo",):
    if _p not in sys.path:
        sys.path.insert(0, _p)

import math

import ml_dtypes
import numpy as np

B, C, H, W = 4, 256, 48, 48
HEADS, HD, HALF = 8, 32, 8
NCORES = 8
RQ = 24                    # query rows per core
RB = 32                    # k/v band rows (24 + 8 halo)
NQ = W * RQ                # 1152 query tokens (w-major)
NB = W * RB                # 1536 band tokens (w-major)
NQB = NQ // 128            # 9 query blocks
NCK = NB // 128            # 12 key chunks (4 w-cols x 32 h each)
SCALE = 1.0 / math.sqrt(HD)

bf16 = ml_dtypes.bfloat16


def _pairs():
    """(qblock, chunk) pairs with their 32-aligned q-token overlap.

    Chunk ck covers w-cols [4ck, 4ck+4); its +-8 w-window reaches q
    tokens [96ck-192, 96ck+288). 96 and 128 are both multiples of 32,
    so overlap bounds are 32-aligned automatically.
    """
    ps, moff = [], 0
    for qb in range(NQB):
        for ck in range(NCK):
            a = max(128 * qb, 96 * ck - 192)
            b = min(128 * qb + 128, 96 * ck + 288, NQ)
            if a < b:
                ps.append((qb, ck, a, b, moff))
                moff += b - a
    return ps, moff


PAIRS, MTOT = _pairs()


def _pieces(a, w):
    """Split [a, a+w) into base-aligned pieces (PSUM partition rule:
    an access at base b may span at most the aligned block size)."""
    out, b, end = [], a, a + w
    while b < end:
        for s in (128, 64, 32):
            if b % s == 0 and b + s <= end:
                out.append((b, s))
                b += s
                break
        else:
            raise AssertionError((a, w))
    return out


# pairs whose mask-multiply runs on GpSimd instead of DVE (load balance)
POOL_MASK = frozenset()

_PROG = None


def _build_program():
    import concourse.bass as bass  # noqa: F401
    import concourse.mybir as mybir
    import concourse.tile as tile
    from concourse import bacc

    f32 = mybir.dt.float32
    bft = mybir.dt.bfloat16
    AF = mybir.ActivationFunctionType
    OP = mybir.AluOpType

    nc = bacc.Bacc(
        "TRN2", target_bir_lowering=False, debug=False, num_devices=NCORES
    )

    def din(name, shape, dt=bft):
        return nc.dram_tensor(name, shape, dt, kind="ExternalInput").ap()

    xt_d = din("xT", [C, NB])
    xres_d = din("xres", [C, NQ], f32)
    wqk_d = din("wqkT", [C, 2 * C])
    wv_d = din("wvT", [C, C])
    wo_d = din("woT", [C, C])
    wc_d = din("wcT", [C, C])
    bqk_d = din("bqk", [128, 4], f32)
    bv_d = din("bvrep", [128, C], f32)
    bc_d = din("bcrep", [128, 2], f32)
    mask_d = din("masks", [128, MTOT])
    id_d = din("ident", [128, 128])
    out_d = nc.dram_tensor("out", [C, NQ], f32, kind="ExternalOutput").ap()

    with tile.TileContext(nc) as tc:
        import contextlib

        ctx = contextlib.ExitStack()
        with ctx:
            cpool = ctx.enter_context(tc.tile_pool(name="const", bufs=1))
            qkpool = ctx.enter_context(tc.tile_pool(name="qk", bufs=1))
            vpool = ctx.enter_context(tc.tile_pool(name="v", bufs=1))
            epool = ctx.enter_context(tc.tile_pool(name="exp", bufs=4))
            apool = ctx.enter_context(tc.tile_pool(name="attn", bufs=4))
            rpool = ctx.enter_context(tc.tile_pool(name="recip", bufs=3))
            opool = ctx.enter_context(tc.tile_pool(name="outb", bufs=3))
            psSC = ctx.enter_context(
                tc.tile_pool(name="psSC", bufs=2, space="PSUM")
            )
            psPV = ctx.enter_context(
                tc.tile_pool(name="psPV", bufs=2, space="PSUM")
            )
            psT = ctx.enter_context(
                tc.tile_pool(name="psT", bufs=1, space="PSUM")
            )
            psP = ctx.enter_context(
                tc.tile_pool(name="psP", bufs=1, space="PSUM")
            )

            # ---- constants / inputs to SBUF ----
            xt = [cpool.tile([128, NB], bft, tag=f"xt{t}", name=f"xt{t}") for t in range(2)]
            for t in range(2):
                nc.sync.dma_start(xt[t][:], xt_d[128 * t : 128 * t + 128, :])
            wqk = [cpool.tile([128, 2 * C], bft, tag=f"wqk{t}", name=f"wqk{t}") for t in range(2)]
            wv = [cpool.tile([128, C], bft, tag=f"wv{t}", name=f"wv{t}") for t in range(2)]
            wo = [cpool.tile([128, C], bft, tag=f"wo{t}", name=f"wo{t}") for t in range(2)]
            wc = [cpool.tile([128, C], bft, tag=f"wc{t}", name=f"wc{t}") for t in range(2)]
            for t in range(2):
                sl = slice(128 * t, 128 * t + 128)
                nc.sync.dma_start(wqk[t][:], wqk_d[sl, :])
                nc.sync.dma_start(wv[t][:], wv_d[sl, :])
                nc.sync.dma_start(wo[t][:], wo_d[sl, :])
                nc.sync.dma_start(wc[t][:], wc_d[sl, :])
            bqk = cpool.tile([128, 4], f32, tag="bqk")
            bvr = cpool.tile([128, C], f32, tag="bvr")
            bcr = cpool.tile([128, 2], f32, tag="bcr")
            nc.sync.dma_start(bqk[:], bqk_d[:])
            nc.sync.dma_start(bvr[:], bv_d[:])
            nc.sync.dma_start(bcr[:], bc_d[:])
            ident = cpool.tile([128, 128], bft, tag="ident")
            nc.sync.dma_start(ident[:], id_d[:])
            zrow = cpool.tile([1, 512], bft, tag="zrow")
            nc.vector.memset(zrow[:], 0.0)
            msk = cpool.tile([128, MTOT], bft, tag="msk")
            nc.sync.dma_start(msk[:], mask_d[:])
            xres = [cpool.tile([128, NQ], f32, tag=f"xres{t}", name=f"xres{t}") for t in range(2)]
            for t in range(2):
                nc.sync.dma_start(xres[t][:], xres_d[128 * t : 128 * t + 128, :])

            # ---- phase 1: qk projection ----
            # qq[quad] [128, NQ]: q channels of heads 4*quad..4*quad+4,
            # q tokens w-major (strided view of the band).
            # kk[quad] [128, NB]: k channels, all band tokens.
            qq = [qkpool.tile([128, NQ], bft, tag=f"qq{i}", name=f"qq{i}") for i in range(2)]
            kk = [qkpool.tile([128, NB], bft, tag=f"kk{i}", name=f"kk{i}") for i in range(2)]
            vt = [vpool.tile([128, HEADS * 33], bft, tag=f"v{i}", name=f"v{i}") for i in range(NCK)]

            def q_proj(quad, t):
                ps = psP.tile([128, 512], f32, tag="ps", name="ps")
                for cc in range(2):
                    rhs = xt[cc][:].rearrange("p (w h) -> p w h", h=RB)[
                        :, 16 * t : 16 * t + 16, 0:RQ
                    ]
                    nc.tensor.matmul(
                        ps[:, 0:384],
                        lhsT=wqk[cc][:, 128 * quad : 128 * quad + 128],
                        rhs=rhs,
                        start=(cc == 0),
                        stop=(cc == 1),
                    )
                nc.vector.tensor_scalar_add(
                    qq[quad][:, 384 * t : 384 * t + 384],
                    ps[:, 0:384],
                    bqk[:, quad : quad + 1],
                )

            def k_proj(quad, t):
                ps = psP.tile([128, 512], f32, tag="ps", name="ps")
                for cc in range(2):
                    nc.tensor.matmul(
                        ps[:],
                        lhsT=wqk[cc][:, 256 + 128 * quad : 256 + 128 * quad + 128],
                        rhs=xt[cc][:, 512 * t : 512 * t + 512],
                        start=(cc == 0),
                        stop=(cc == 1),
                    )
                nc.vector.tensor_scalar_add(
                    kk[quad][:, 512 * t : 512 * t + 512],
                    ps[:],
                    bqk[:, 2 + quad : 3 + quad],
                )

            def v_proj(i):
                ps = psP.tile([128, 512], f32, tag="ps", name="ps")
                for cc in range(2):
                    nc.tensor.matmul(
                        ps[:, 0:C],
                        lhsT=xt[cc][:, 128 * i : 128 * i + 128],
                        rhs=wv[cc][:],
                        start=(cc == 0),
                        stop=(cc == 1),
                    )
                va = vt[i][:].rearrange("p (h x) -> p h x", x=33)
                nc.vector.tensor_add(
                    va[:, :, 0:32],
                    ps[:, 0:C].rearrange("p (h d) -> p h d", d=32),
                    bvr[:].rearrange("p (h d) -> p h d", d=32),
                )
                nc.gpsimd.memset(va[:, :, 32:33], 1.0)

            # interleave so attention can start early: tile-major order
            for t in range(3):
                for quad in range(2):
                    q_proj(quad, t)
                    k_proj(quad, t)
                for i in range(4 * t, 4 * t + 4):
                    v_proj(i)

            # All-heads-at-partition-0 copies of q/k (heads along the free
            # dim) via partition-shifting DMAs. Score matmuls then all use
            # row group 0: mixing different tile_position row groups within
            # one PSUM bank is illegal.
            q32 = qkpool.tile([32, HEADS * NQ], bft, tag="q32")
            k32 = qkpool.tile([32, HEADS * NB], bft, tag="k32")
            q32v = q32[:].rearrange("p (h n) -> p h n", n=NQ)
            k32v = k32[:].rearrange("p (h n) -> p h n", n=NB)
            for h in range(HEADS):
                quad, s = h // 4, 32 * (h % 4)
                nc.sync.dma_start(q32v[:, h, :], qq[quad][s : s + 32, :])
                nc.sync.dma_start(k32v[:, h, :], kk[quad][s : s + 32, :])

            # ---- phase 2: attention ----
            oT = [cpool.tile([128, NQ], bft, tag=f"oT{g}", name=f"oT{g}") for g in range(2)]
            res = [cpool.tile([128, NQ], bft, tag=f"res{t}", name=f"res{t}") for t in range(2)]

            by_qb = {}
            for idx, (qb, ck, a, b, moff) in enumerate(PAIRS):
                by_qb.setdefault(qb, []).append((idx, ck, a, b, moff))

            for qb in range(NQB):
                plist = by_qb[qb]
                pv = psPV.tile([128, 512], f32, tag="pv", name="pv")
                nc.tensor.matmul(
                    pv[:, 0 : HEADS * 33],
                    lhsT=zrow[:, 0:128],
                    rhs=zrow[:, 0 : HEADS * 33],
                    start=True,
                    stop=False,
                    skip_group_check=True,
                )
                pv_v = pv[:, 0 : HEADS * 33].rearrange("p (h x) -> p h x", x=33)
                for j, (idx, ck, a, b, moff) in enumerate(plist):
                    aoff = a - 128 * qb
                    w_ = b - a
                    last_pair = j == len(plist) - 1
                    sc = psSC.tile([128, 1024], f32, tag="sc", name="sc")
                    for h in range(HEADS):
                        nc.tensor.matmul(
                            sc[:, 128 * h + aoff : 128 * h + aoff + w_],
                            lhsT=k32v[:, h, 128 * ck : 128 * ck + 128],
                            rhs=q32v[:, h, a:b],
                            start=True,
                            stop=True,
                        )
                    sc_v = sc[:].rearrange("p (h q) -> p h q", q=128)[
                        :, :, aoff : aoff + w_
                    ]
                    ex = epool.tile([128, 1024], bft, tag="ex", name="ex")
                    ex_v = ex[:].rearrange("p (h q) -> p h q", q=128)[
                        :, :, aoff : aoff + w_
                    ]
                    nc.scalar.activation(ex_v, sc_v, AF.Exp, scale=SCALE)
                    ma = apool.tile([128, 1024], bft, tag="ma", name="ma")
                    ma_v = ma[:].rearrange("p (h q) -> p h q", q=128)
                    mk = msk[:, moff : moff + w_]
                    eng = nc.gpsimd if idx in POOL_MASK else nc.vector
                    eng.tensor_mul(
                        ma_v[:, :, aoff : aoff + w_],
                        ex_v,
                        mk[:, None, :].broadcast_to([128, HEADS, w_]),
                    )
                    for h in range(HEADS):
                        pcs = _pieces(aoff, w_)
                        for pi, (pb, pw) in enumerate(pcs):
                            nc.tensor.matmul(
                                pv[pb : pb + pw, 33 * h : 33 * h + 33],
                                lhsT=ma_v[:, h, pb : pb + pw],
                                rhs=vt[ck][:].rearrange(
                                    "p (h x) -> p h x", x=33
                                )[:, h, :],
                                start=False,
                                stop=(
                                    last_pair
                                    and h == HEADS - 1
                                    and pi == len(pcs) - 1
                                ),
                                skip_group_check=True,
                                tile_position=(0, pb),
                            )
                # ---- normalize + transpose this qblock ----
                sums = rpool.tile([128, 8], f32, tag="sums", name="sums")
                nc.vector.tensor_copy(
                    sums[:].rearrange("p (h x) -> p h x", x=1),
                    pv_v[:, :, 32:33],
                )
                rc = rpool.tile([128, 8], f32, tag="rc", name="rc")
                nc.vector.reciprocal(rc[:], sums[:])
                ot = opool.tile([128, C], bft, tag="ot", name="ot")
                nc.vector.tensor_mul(
                    ot[:].rearrange("p (h d) -> p h d", d=32),
                    pv_v[:, :, 0:32],
                    rc[:]
                    .rearrange("p (h x) -> p h x", x=1)
                    .broadcast_to([128, 8, 32]),
                )
                pst = psT.tile([128, 1024], bft, tag="pst", name="pst")
                for cc in range(2):
                    nc.tensor.transpose(
                        pst[:, 128 * cc : 128 * cc + 128],
                        ot[:, 128 * cc : 128 * cc + 128],
                        ident[:],
                    )
                    nc.vector.tensor_copy(
                        oT[cc][:, 128 * qb : 128 * qb + 128],
                        pst[:, 128 * cc : 128 * cc + 128],
                    )

                # ---- projections per 384-token group ----
                if qb % 3 == 2:
                    g3 = qb // 3
                    n0 = 384 * g3
                    for oc in range(2):
                        ps = psP.tile([128, 512], f32, tag="ps", name="ps")
                        for cc in range(2):
                            nc.tensor.matmul(
                                ps[:, 0:384],
                                lhsT=wo[cc][:, 128 * oc : 128 * oc + 128],
                                rhs=oT[cc][:, n0 : n0 + 384],
                                start=(cc == 0),
                                stop=(cc == 1),
                            )
                        nc.vector.tensor_add(
                            res[oc][:, n0 : n0 + 384],
                            ps[:, 0:384],
                            xres[oc][:, n0 : n0 + 384],
                        )
                    for oc in range(2):
                        ps = psP.tile([128, 512], f32, tag="ps", name="ps")
                        for cc in range(2):
                            nc.tensor.matmul(
                                ps[:, 0:384],
                                lhsT=wc[cc][:, 128 * oc : 128 * oc + 128],
                                rhs=res[cc][:, n0 : n0 + 384],
                                start=(cc == 0),
                                stop=(cc == 1),
                            )
                        ob = opool.tile([128, 384], f32, tag="ob", name="ob")
                        nc.vector.tensor_scalar(
                            ob[:],
                            ps[:, 0:384],
                            bcr[:, oc : oc + 1],
                            0.0,
                            OP.add,
                            OP.max,
                        )
                        nc.sync.dma_start(
                            out_d[128 * oc : 128 * oc + 128, n0 : n0 + 384],
                            ob[:],
                        )

    nc.compile()
    return nc


def _get_program():
    global _PROG
    if _PROG is None:
        _PROG = _build_program()
    return _PROG


_MASK_CACHE = {}


def _masks() -> np.ndarray:
    """[128, MTOT] binary window masks, shared by every core.

    Per (qblock, chunk) pair: key partition p -> (w_k, h_k) =
    (4ck + p//32, p%32); q token t -> (t//24, t%24)."""
    if "m" in _MASK_CACHE:
        return _MASK_CACHE["m"]
    m = np.zeros((128, MTOT), bf16)
    p = np.arange(128)
    for qb, ck, a, b, moff in PAIRS:
        wk, hk = 4 * ck + p // 32, p % 32
        t = np.arange(a, b)
        wq, hq = t // RQ, t % RQ
        m[:, moff : moff + b - a] = (
            (np.abs(wk[:, None] - wq[None, :]) <= HALF)
            & (np.abs(hk[:, None] - hq[None, :]) <= HALF)
        ).astype(bf16)
    _MASK_CACHE["m"] = m
    return m


def _prep_core_inputs(core, x, in_proj_w, in_proj_b, out_w, out_b, conv_w, conv_b):
    b, half = core // 2, core % 2
    ximg = x[b]
    if half == 1:
        ximg = ximg[:, ::-1, :]  # row-flip: half-1 becomes half-0 geometry
    band = ximg[:, :RB, :].transpose(0, 2, 1)  # [C, W, RB] w-major
    xq = ximg[:, :RQ, :].transpose(0, 2, 1)    # [C, W, RQ]
    xres = (xq.reshape(C, NQ) + out_b[:, None]).astype(np.float32)
    return {
        "xT": np.ascontiguousarray(band.reshape(C, NB)).astype(bf16),
        "xres": np.ascontiguousarray(xres),
        "wqkT": np.ascontiguousarray(in_proj_w[: 2 * C].T).astype(bf16),
        "wvT": np.ascontiguousarray(in_proj_w[2 * C :].T).astype(bf16),
        "woT": np.ascontiguousarray(out_w.T).astype(bf16),
        "wcT": np.ascontiguousarray(conv_w.T).astype(bf16),
        "bqk": np.ascontiguousarray(
            in_proj_b[: 2 * C].reshape(4, 128).T
        ).astype(np.float32),
        "bvrep": np.broadcast_to(in_proj_b[2 * C :], (128, C)).astype(np.float32).copy(),
        "bcrep": np.ascontiguousarray(conv_b.reshape(2, 128).T).astype(np.float32),
        "masks": _masks(),
        "ident": np.eye(128, dtype=bf16),
    }


def kernel(**inputs):
    from concourse.bass_utils import run_bass_kernel_spmd

    args = {k: np.asarray(v) for k, v in inputs.items()}
    nc = _get_program()
    in_maps = [_prep_core_inputs(core, **args) for core in range(NCORES)]
    res = run_bass_kernel_spmd(nc, in_maps, core_ids=list(range(NCORES)))
    out = np.zeros((B, C, H, W), np.float32)
    for core in range(NCORES):
        b, half = core // 2, core % 2
        o = res.results[core]["out"].reshape(C, W, RQ).transpose(0, 2, 1)
        if half == 1:
            o = o[:, ::-1, :]  # undo the row flip
            out[b][:, RQ:, :] = o
        else:
            out[b][:, :RQ, :] = o
    return out
